# revision 4
# baseline (speedup 1.0000x reference)
"""DropoutDPP eval-path kernel for 8 Trainium2 NeuronCores.

The reference accumulates Bernoulli keep-masks (fixed RNG key 42, independent
of x) over the hidden dim until >=40% of neurons are nonzero, then computes
x * sum_mask / i.  The mask state is a deterministic constant, so it is
computed host-side (same jax threefry bits as the reference) and folded into a
single [hidden] scale vector.  The heavy, memory-bound part — scaling the
[4, 4096, 4096] tensor along its last dim — runs on 8 cores, data-parallel
over rows.

The on-device kernel is hand-scheduled raw Bass (this toolchain's TT struct
encodes a single sync wait, which rules out Tile's multi-wait scheduling):
SP issues loads, DVE multiplies in place, ACT issues stores; three 4MB SBUF
slots rotate.
"""

import numpy as np

_B, _S, _H = 4, 4096, 4096
_N_CORES = 8
_ROWS = _B * _S                       # 16384
_ROWS_PER_CORE = _ROWS // _N_CORES    # 2048
_P = 0.9
_MAX_N = 100
_MAX_FRAC = 0.4

_J = 2                                # 128-row blocks per SBUF tile (4MB tiles)
_ROWS_PER_TILE = 128 * _J
_N_TILES = _ROWS_PER_CORE // _ROWS_PER_TILE
_N_SLOTS = 3

_cache: dict = {}


def _compute_scale() -> np.ndarray:
    """Replicate reference._accumulate_masks exactly (threefry is
    backend/platform deterministic), returning sum_mask / i as float32."""
    if "scale" in _cache:
        return _cache["scale"]
    import jax
    import jax.numpy as jnp

    cpu = jax.devices("cpu")[0]
    with jax.default_device(cpu):
        key = jax.random.key(42)
        key, k0 = jax.random.split(key)
        sum_mask = (jax.random.uniform(k0, (_H,)) >= _P).astype(jnp.float32)
        i = 1
        while i < _MAX_N and float(
            jnp.mean((sum_mask != 0).astype(jnp.float32))
        ) < _MAX_FRAC:
            key, k = jax.random.split(key)
            sum_mask = sum_mask + (jax.random.uniform(k, (_H,)) >= _P).astype(
                jnp.float32
            )
            i += 1
    scale = np.asarray(sum_mask, dtype=np.float32) / np.float32(i)
    _cache["scale"] = scale
    return scale


def _build_nc():
    if "nc" in _cache:
        return _cache["nc"]
    import concourse.bass as bass
    import concourse.mybir as mybir
    from contextlib import ExitStack

    nc = bass.Bass(trn_type="TRN2")
    x = nc.dram_tensor(
        "x", [_ROWS_PER_CORE, _H], mybir.dt.float32, kind="ExternalInput"
    )
    scale = nc.dram_tensor(
        "scale", [128, _H], mybir.dt.float32, kind="ExternalInput"
    )
    y = nc.dram_tensor(
        "y", [_ROWS_PER_CORE, _H], mybir.dt.float32, kind="ExternalOutput"
    )

    xv = x[:, :].rearrange("(n j p) h -> n p j h", j=_J, p=128)
    yv = y[:, :].rearrange("(n j p) h -> n p j h", j=_J, p=128)

    with ExitStack() as ctx:
        scale_sb = ctx.enter_context(
            nc.sbuf_tensor("scale_sb", [128, _H], mybir.dt.float32)
        )
        slots = [
            ctx.enter_context(
                nc.sbuf_tensor(f"slot{s}", [128, _J, _H], mybir.dt.float32)
            )
            for s in range(_N_SLOTS)
        ]
        load_sem = ctx.enter_context(nc.semaphore("load_sem"))
        store_sem = ctx.enter_context(nc.semaphore("store_sem"))
        mul_sem = ctx.enter_context(nc.semaphore("mul_sem"))
        block = ctx.enter_context(nc.Block())

        @block.sync
        def _(sync):
            # scale first, then the x tiles; slot reuse gated on stores
            sync.dma_start(out=scale_sb[:, :], in_=scale[:, :]).then_inc(
                load_sem, 16
            )
            for i in range(_N_TILES):
                if i >= _N_SLOTS:
                    sync.wait_ge(store_sem, 16 * (i - _N_SLOTS + 1))
                sync.dma_start(
                    out=slots[i % _N_SLOTS][:, :, :], in_=xv[i]
                ).then_inc(load_sem, 16)

        @block.vector
        def _(vector):
            for i in range(_N_TILES):
                # scale + loads 0..i complete
                vector.wait_ge(load_sem, 16 * (i + 2))
                t = slots[i % _N_SLOTS]
                for jj in range(_J):
                    inst = vector.tensor_mul(
                        out=t[:, jj, :], in0=t[:, jj, :], in1=scale_sb[:, :]
                    )
                    if jj == _J - 1:
                        inst.then_inc(mul_sem, 1)

        @block.scalar
        def _(scalar):
            for i in range(_N_TILES):
                scalar.wait_ge(mul_sem, i + 1)
                scalar.dma_start(
                    out=yv[i], in_=slots[i % _N_SLOTS][:, :, :]
                ).then_inc(store_sem, 16)
            # all output bytes landed before the program ends
            scalar.wait_ge(store_sem, 16 * _N_TILES)

    _cache["nc"] = nc
    return nc


def _run(x: np.ndarray, trace: bool = False, trace_cores=None):
    """Returns (full_output, BassKernelResults)."""
    from concourse.bass_utils import run_bass_kernel_spmd

    nc = _build_nc()
    scale = _compute_scale()
    scale_bc = np.ascontiguousarray(
        np.broadcast_to(scale[None, :], (128, _H))
    )
    xf = np.ascontiguousarray(x, dtype=np.float32).reshape(_ROWS, _H)
    in_maps = [
        {"x": xf[c * _ROWS_PER_CORE : (c + 1) * _ROWS_PER_CORE], "scale": scale_bc}
        for c in range(_N_CORES)
    ]
    res = run_bass_kernel_spmd(
        nc,
        in_maps,
        core_ids=list(range(_N_CORES)),
        trace=trace,
        trace_cores=trace_cores,
    )
    out = np.concatenate([r["y"] for r in res.results], axis=0)
    return out.reshape(_B, _S, _H), res


def kernel(**inputs) -> np.ndarray:
    out, _ = _run(np.asarray(inputs["x"]))
    return out


# revision 13
# speedup vs baseline: 1.2056x; 1.2056x over previous
"""DropoutDPP eval-path kernel for 8 Trainium2 NeuronCores.

The reference accumulates Bernoulli keep-masks (fixed RNG key 42, independent
of x) over the hidden dim until >=40% of neurons are nonzero, then computes
x * sum_mask / i.  The mask state is a deterministic constant, so it is
computed host-side (same jax threefry bits as the reference) and folded into a
single [hidden] scale vector.  The heavy, memory-bound part — scaling the
[4, 4096, 4096] tensor along its last dim — runs on 8 cores, data-parallel
over rows.

The on-device kernel is hand-scheduled raw Bass (this toolchain's TT struct
encodes a single sync wait, which rules out Tile's multi-wait scheduling):
SP issues loads, DVE multiplies in place, ACT issues stores; three 4MB SBUF
slots rotate.
"""

import numpy as np

_B, _S, _H = 4, 4096, 4096
_N_CORES = 8
_ROWS = _B * _S                       # 16384
_ROWS_PER_CORE = _ROWS // _N_CORES    # 2048
_P = 0.9
_MAX_N = 100
_MAX_FRAC = 0.4

_J = 1                                # 128-row blocks per SBUF tile (2MB tiles)
_ROWS_PER_TILE = 128 * _J
_N_TILES = _ROWS_PER_CORE // _ROWS_PER_TILE
_N_SLOTS = 8

_cache: dict = {}


def _compute_scale() -> np.ndarray:
    """Replicate reference._accumulate_masks exactly (threefry is
    backend/platform deterministic), returning sum_mask / i as float32."""
    if "scale" in _cache:
        return _cache["scale"]
    import jax
    import jax.numpy as jnp

    cpu = jax.devices("cpu")[0]
    with jax.default_device(cpu):
        key = jax.random.key(42)
        key, k0 = jax.random.split(key)
        sum_mask = (jax.random.uniform(k0, (_H,)) >= _P).astype(jnp.float32)
        i = 1
        while i < _MAX_N and float(
            jnp.mean((sum_mask != 0).astype(jnp.float32))
        ) < _MAX_FRAC:
            key, k = jax.random.split(key)
            sum_mask = sum_mask + (jax.random.uniform(k, (_H,)) >= _P).astype(
                jnp.float32
            )
            i += 1
    scale = np.asarray(sum_mask, dtype=np.float32) / np.float32(i)
    _cache["scale"] = scale
    return scale


def _build_nc():
    if "nc" in _cache:
        return _cache["nc"]
    import concourse.bass as bass
    import concourse.mybir as mybir
    from contextlib import ExitStack

    nc = bass.Bass(trn_type="TRN2")
    x = nc.dram_tensor(
        "x", [_ROWS_PER_CORE, _H], mybir.dt.float32, kind="ExternalInput"
    )
    scale = nc.dram_tensor(
        "scale", [128, _H], mybir.dt.float32, kind="ExternalInput"
    )
    y = nc.dram_tensor(
        "y", [_ROWS_PER_CORE, _H], mybir.dt.float32, kind="ExternalOutput"
    )

    xv = x[:, :].rearrange("(n p) h -> n p h", p=128)
    yv = y[:, :].rearrange("(n p) h -> n p h", p=128)

    with ExitStack() as ctx:
        scale_sb = ctx.enter_context(
            nc.sbuf_tensor("scale_sb", [128, _H], mybir.dt.float32)
        )
        slots = [
            ctx.enter_context(
                nc.sbuf_tensor(f"slot{s}", [128, _H], mybir.dt.float32)
            )
            for s in range(_N_SLOTS)
        ]
        # One semaphore per slot: each slot has at most one outstanding DMA
        # at a time (load +16, store +16 → +32 per slot cycle), making the
        # wait thresholds exact.  A single shared DMA sem would race: the 16
        # SDMA engines increment independently per transfer, so "sem >=
        # 16*(i+1)" does not imply transfers 0..i all completed.
        slot_sems = [
            ctx.enter_context(nc.semaphore(f"slot_sem{s}"))
            for s in range(_N_SLOTS)
        ]
        mul_sem = ctx.enter_context(nc.semaphore("mul_sem"))
        sc_sem = ctx.enter_context(nc.semaphore("sc_sem"))
        block = ctx.enter_context(nc.Block())

        n_cycles = _N_TILES // _N_SLOTS

        @block.gpsimd
        def _(gpsimd):
            # scale broadcast rows via SWDGE — off the load/store HWDGE rings
            gpsimd.dma_start(out=scale_sb[:, :], in_=scale[:, :]).then_inc(
                sc_sem, 16
            )

        @block.sync
        def _(sync):
            for i in range(_N_TILES):
                s, cyc = i % _N_SLOTS, i // _N_SLOTS
                if cyc > 0:
                    sync.wait_ge(slot_sems[s], 32 * cyc)  # prev store landed
                sync.dma_start(out=slots[s][:, :], in_=xv[i]).then_inc(
                    slot_sems[s], 16
                )

        @block.vector
        def _(vector):
            vector.wait_ge(sc_sem, 16)
            for i in range(_N_TILES):
                s, cyc = i % _N_SLOTS, i // _N_SLOTS
                vector.wait_ge(slot_sems[s], 32 * cyc + 16)  # this load landed
                t = slots[s]
                vector.tensor_mul(
                    out=t[:, :], in0=t[:, :], in1=scale_sb[:, :]
                ).then_inc(mul_sem, 1)

        @block.scalar
        def _(scalar):
            for i in range(_N_TILES):
                s = i % _N_SLOTS
                scalar.wait_ge(mul_sem, i + 1)
                scalar.dma_start(out=yv[i], in_=slots[s][:, :]).then_inc(
                    slot_sems[s], 16
                )
            # all output bytes landed before the program ends
            for s in range(_N_SLOTS):
                scalar.wait_ge(slot_sems[s], 32 * n_cycles)

    _cache["nc"] = nc
    return nc


def _run(x: np.ndarray, trace: bool = False, trace_cores=None):
    """Returns (full_output, BassKernelResults)."""
    from concourse.bass_utils import run_bass_kernel_spmd

    nc = _build_nc()
    scale_bc = np.ascontiguousarray(
        np.broadcast_to(_compute_scale()[None, :], (128, _H))
    )
    xf = np.ascontiguousarray(x, dtype=np.float32).reshape(_ROWS, _H)
    in_maps = [
        {"x": xf[c * _ROWS_PER_CORE : (c + 1) * _ROWS_PER_CORE], "scale": scale_bc}
        for c in range(_N_CORES)
    ]
    res = run_bass_kernel_spmd(
        nc,
        in_maps,
        core_ids=list(range(_N_CORES)),
        trace=trace,
        trace_cores=trace_cores,
    )
    out = np.concatenate([r["y"] for r in res.results], axis=0)
    return out.reshape(_B, _S, _H), res


def kernel(**inputs) -> np.ndarray:
    out, _ = _run(np.asarray(inputs["x"]))
    return out
